# revision 15
# baseline (speedup 1.0000x reference)
"""Trainium2 Bass kernel for the fused 3-modality attention + FFN + softmax model.

Layout strategy: pure data parallel over 8 NeuronCores (batch sharded), all
activations kept FEATURE-MAJOR on chip ([1024 feats = 8 chunks x 128
partitions, tokens in the free dim]) so no on-device transposes are needed.
All GEMMs run in bf16 (1 cycle/row on the PE) accumulating fp32 in PSUM.
LayerNorm reductions over features are ones-vector matmuls on the PE;
per-token scalars are broadcast back across partitions with K=1 expand
matmuls. Host-side prep: transpose activations to [DIM, B] bf16, pre-scale
Wg by 1/3 (modality mean) and Wq/bq by 1/sqrt(HD) (attention scale).
"""

import numpy as np
import ml_dtypes

import concourse.bacc as bacc
import concourse.bass as bass
import concourse.mybir as mybir
import concourse.tile as tile

B, DIM, H, FFN, HD = 16384, 1024, 16, 4096, 64
NCORES = 8
TPC = B // NCORES          # tokens per core
TB = 512                   # token block (matmul moving dim)
KC = DIM // 128            # 8 feature chunks
MC1 = FFN // 128           # 32 ffn chunks
EPS = 1e-5

BF16 = mybir.dt.bfloat16
F32 = mybir.dt.float32
F32R = mybir.dt.float32r
AF = mybir.ActivationFunctionType


def _ln_apply(nc, pp, wk, src_f32, out_bf, g, be, C):
    """LayerNorm over features (partition x chunk axis); src modified in place.

    src_f32: [128, KC*TB] fp32 tile, out_bf: [128, KC*TB] bf16 tile.
    Per-token scalars live as rows of one packed [8, TB] fp32 tile.
    """
    v, s, te = nc.vector, nc.scalar, nc.tensor
    pr1 = pp.tile([16, TB], F32, tag="red", bufs=3, name="pr1")
    for kc in range(KC):
        te.matmul(pr1[0:1, :], C["onec"][:],
                  src_f32[:, kc * TB:(kc + 1) * TB],
                  start=(kc == 0), stop=(kc == KC - 1))
    sq = wk.tile([128, KC * TB], BF16, tag="qb", bufs=1, name="sq")
    s.activation(sq[:], src_f32[:], AF.Square)
    pr2 = pp.tile([16, TB], F32, tag="red", bufs=3, name="pr2")
    for kc in range(KC):
        te.matmul(pr2[0:1, :], C["onecb"][:], sq[:, kc * TB:(kc + 1) * TB],
                  start=(kc == 0), stop=(kc == KC - 1))
    # Each per-token scalar gets its own tile: SBUF APs must start at a
    # 32-aligned partition AND both SB inputs of any tensor-tensor op must
    # share the same base partition -> keep everything at base 0.
    mu = wk.tile([1, TB], F32, tag="ln_mu", bufs=1, name="mu")[:]
    ex2 = wk.tile([1, TB], F32, tag="ln_ex2", bufs=1, name="ex2")[:]
    mu2 = wk.tile([1, TB], F32, tag="ln_mu2", bufs=1, name="mu2")[:]
    var = wk.tile([1, TB], F32, tag="ln_var", bufs=1, name="var")[:]
    sd = wk.tile([1, TB], F32, tag="ln_sd", bufs=1, name="sd")[:]
    rs = wk.tile([1, TB], F32, tag="ln_rs", bufs=1, name="rs")[:]
    s.activation(mu, pr1[0:1, :], AF.Copy, scale=1.0 / DIM)
    s.activation(ex2, pr2[0:1, :], AF.Copy, scale=1.0 / DIM)
    s.activation(mu2, mu, AF.Square)
    v.tensor_sub(var, ex2, mu2)
    s.activation(sd, var, AF.Sqrt, bias=C["epsc"][:])
    v.reciprocal(rs, sd)
    pmu = pp.tile([128, TB], F32, tag="acc", bufs=4, name="pmu")
    te.matmul(pmu[:], C["oner"][:], mu, start=True, stop=True)
    prs = pp.tile([128, TB], F32, tag="acc", bufs=4, name="prs")
    te.matmul(prs[:], C["oner"][:], rs, start=True, stop=True)
    mus = wk.tile([128, TB], F32, tag="mus", bufs=1, name="mus")
    v.tensor_copy(mus[:], pmu[:])
    rss = wk.tile([128, TB], F32, tag="rss", bufs=1, name="rss")
    v.tensor_copy(rss[:], prs[:])
    for kc in range(KC):
        sl = src_f32[:, kc * TB:(kc + 1) * TB]
        v.tensor_sub(sl, sl, mus[:])
        v.tensor_mul(sl, sl, rss[:])
        s.activation(out_bf[:, kc * TB:(kc + 1) * TB], sl, AF.Identity,
                     scale=g[:, kc:kc + 1], bias=be[:, kc:kc + 1])


def _emit(nc, tc, io, tpc):
    nblk = tpc // TB
    v, s, te = nc.vector, nc.scalar, nc.tensor

    with (
        tc.tile_pool(name="consts", bufs=1) as cp,
        tc.tile_pool(name="psum", bufs=1, space="PSUM") as pp,
    ):
        # ---- constants / small params ----
        C = {}
        for name, shape, dtype in (
            ("Ssel", [128, 128], BF16), ("Eexp", [16, 1024], BF16),
            ("onec", [128, 1], F32), ("onecb", [128, 1], BF16),
            ("oner", [1, 128], F32),
            ("bg", [128, KC], F32), ("bq", [128, KC], F32),
            ("bk", [128, KC], F32), ("bv", [128, KC], F32),
            ("b1", [128, MC1], F32), ("b2", [128, KC], F32),
            ("g1", [128, KC], F32), ("be1", [128, KC], F32),
            ("g2", [128, KC], F32), ("be2", [128, KC], F32),
            ("Wwt", [128, 3 * KC], BF16), ("bwc", [1, 3], F32),
            ("epsc", [1, 1], F32),
        ):
            t = cp.tile(shape, dtype, name=f"c_{name}")
            nc.sync.dma_start(out=t[:], in_=io[name])
            C[name] = t

        # ---------------- phase A: attention + LN1 ----------------
        with (
            tc.tile_pool(name="wA", bufs=1) as wa,
            tc.tile_pool(name="workA", bufs=1) as wk,
        ):
            wmap = {}
            for wn in ("Wg", "Wq", "Wk", "Wv"):
                wt = wa.tile([128, KC * DIM], BF16, name=f"w_{wn}")
                nc.sync.dma_start(
                    out=wt[:].rearrange("p (c n) -> p c n", n=DIM),
                    in_=io[wn].rearrange("(c p) n -> p c n", p=128))
                wmap[wn] = wt

            def wsl(wn, kc, mc):
                return wmap[wn][:, kc * DIM + mc * 128:kc * DIM + mc * 128 + 128]

            for blk in range(nblk):
                t0 = blk * TB
                ins = {}
                for name in ("m0", "m1", "m2", "dom"):
                    t = wk.tile([128, KC * TB], BF16, tag=f"in_{name}",
                                bufs=1, name=f"{name}_sb")
                    nc.sync.dma_start(
                        out=t[:].rearrange("p (c t) -> p c t", t=TB),
                        in_=io[name].rearrange("(c p) t -> p c t",
                                               p=128)[:, :, t0:t0 + TB])
                    ins[name] = t
                mj = [ins["m0"], ins["m1"], ins["m2"]]
                dom = ins["dom"]

                avg = wk.tile([128, KC * TB], BF16, tag="a4", bufs=2, name="avg")
                v.tensor_add(avg[:], mj[0][:], mj[1][:])
                v.tensor_add(avg[:], avg[:], mj[2][:])

                # global_rep -> qin -> q  (Wg pre-scaled by 1/3)
                gb = wk.tile([128, KC * TB], BF16, tag="a4", bufs=2, name="gb")
                for mc in range(KC):
                    pg = pp.tile([128, TB], F32, tag="acc", bufs=4, name="pg")
                    for kc in range(KC):
                        te.matmul(pg[:], wsl("Wg", kc, mc),
                                  avg[:, kc * TB:(kc + 1) * TB],
                                  start=(kc == 0), stop=(kc == KC - 1))
                    s.activation(gb[:, mc * TB:(mc + 1) * TB], pg[:],
                                 AF.Identity, bias=C["bg"][:, mc:mc + 1])
                qin = wk.tile([128, KC * TB], BF16, tag="a4", bufs=2, name="qin")
                v.tensor_add(qin[:], gb[:], dom[:])

                qb = wk.tile([128, KC * TB], BF16, tag="qb", bufs=1, name="qb")
                for mc in range(KC):
                    pq = pp.tile([128, TB], F32, tag="acc", bufs=4, name="pq")
                    for kc in range(KC):
                        te.matmul(pq[:], wsl("Wq", kc, mc),
                                  qin[:, kc * TB:(kc + 1) * TB],
                                  start=(kc == 0), stop=(kc == KC - 1))
                    s.activation(qb[:, mc * TB:(mc + 1) * TB], pq[:],
                                 AF.Identity, bias=C["bq"][:, mc:mc + 1])

                # scores[h,t] per modality (Wq/bq pre-scaled by 1/sqrt(HD))
                sc = wk.tile([16, 3 * TB], F32, tag="sc", bufs=1,
                             name="sc")
                for j in range(3):
                    for mc in range(KC):
                        pk = pp.tile([128, TB], F32, tag="acc", bufs=4,
                                     name="pk")
                        for kc in range(KC):
                            te.matmul(pk[:], wsl("Wk", kc, mc),
                                      mj[j][:, kc * TB:(kc + 1) * TB],
                                      start=(kc == 0), stop=(kc == KC - 1))
                        kt = wk.tile([128, TB], BF16, tag="kt", bufs=2,
                                     name="kt")
                        s.activation(kt[:], pk[:], AF.Identity,
                                     bias=C["bk"][:, mc:mc + 1])
                        tm = wk.tile([128, TB], BF16, tag="tm", bufs=2,
                                     name="tm")
                        v.tensor_mul(tm[:], qb[:, mc * TB:(mc + 1) * TB], kt[:])
                        ps = pp.tile([16, TB], F32, tag="red", bufs=3,
                                     name="ps")
                        te.matmul(ps[:], C["Ssel"][:, mc * 16:(mc + 1) * 16],
                                  tm[:], start=True, stop=True)
                        scj = sc[:, j * TB:(j + 1) * TB]
                        if mc == 0:
                            v.tensor_copy(scj, ps[:])
                        else:
                            v.tensor_add(scj, scj, ps[:])

                # softmax over the 3 modalities (all tiles at base 0)
                mx = wk.tile([16, TB], F32, tag="mx", bufs=1, name="mx")[:]
                sm = wk.tile([16, TB], F32, tag="sm", bufs=1, name="sm")[:]
                rc = wk.tile([16, TB], F32, tag="rc", bufs=1, name="rc")[:]
                v.tensor_max(mx, sc[:, 0:TB], sc[:, TB:2 * TB])
                v.tensor_max(mx, mx, sc[:, 2 * TB:3 * TB])
                for j in range(3):
                    scj = sc[:, j * TB:(j + 1) * TB]
                    v.tensor_sub(scj, scj, mx)
                ee = wk.tile([16, 3 * TB], F32, tag="ee", bufs=1, name="ee")
                s.activation(ee[:], sc[:], AF.Exp)
                v.tensor_add(sm, ee[:, 0:TB], ee[:, TB:2 * TB])
                v.tensor_add(sm, sm, ee[:, 2 * TB:3 * TB])
                v.reciprocal(rc, sm)
                ab = wk.tile([16, 3 * TB], BF16, tag="ab", bufs=1, name="ab")
                for j in range(3):
                    v.tensor_mul(ab[:, j * TB:(j + 1) * TB],
                                 ee[:, j * TB:(j + 1) * TB], rc)

                # attnout = sum_j bcast(attn_j) * (m_j @ Wv); bv folds to +bv
                xp = wk.tile([128, KC * TB], F32, tag="xp", bufs=1, name="xp")
                for mc in range(KC):
                    acc = wk.tile([128, TB], F32, tag="acc_s", bufs=2,
                                  name="acc")
                    for j in range(3):
                        pv = pp.tile([128, TB], F32, tag="acc", bufs=4,
                                     name="pv")
                        for kc in range(KC):
                            te.matmul(pv[:], wsl("Wv", kc, mc),
                                      mj[j][:, kc * TB:(kc + 1) * TB],
                                      start=(kc == 0), stop=(kc == KC - 1))
                        vt = wk.tile([128, TB], BF16, tag="vt", bufs=2,
                                     name="vt")
                        s.activation(vt[:], pv[:], AF.Copy)
                        pa = pp.tile([128, TB], F32, tag="acc", bufs=4,
                                     name="pa")
                        te.matmul(pa[:], C["Eexp"][:, mc * 128:(mc + 1) * 128],
                                  ab[:, j * TB:(j + 1) * TB],
                                  start=True, stop=True)
                        if j == 0:
                            v.tensor_mul(acc[:], pa[:], vt[:])
                        else:
                            t2 = wk.tile([128, TB], F32, tag="t2", bufs=2,
                                         name="t2")
                            v.tensor_mul(t2[:], pa[:], vt[:])
                            v.tensor_add(acc[:], acc[:], t2[:])
                    tb_ = wk.tile([128, TB], F32, tag="tb", bufs=2, name="tb_")
                    s.activation(tb_[:], acc[:], AF.Identity,
                                 bias=C["bv"][:, mc:mc + 1])
                    v.tensor_add(xp[:, mc * TB:(mc + 1) * TB], tb_[:],
                                 dom[:, mc * TB:(mc + 1) * TB])

                x_bf = wk.tile([128, KC * TB], BF16, tag="xbf", bufs=1,
                               name="x_bf")
                _ln_apply(nc, pp, wk, xp, x_bf, C["g1"], C["be1"], C)
                nc.sync.dma_start(
                    out=io["xs"].rearrange("(c p) t -> p c t",
                                           p=128)[:, :, t0:t0 + TB],
                    in_=x_bf[:].rearrange("p (c t) -> p c t", t=TB))

        # ---------------- phase B: FFN + LN2 + logits ----------------
        with (
            tc.tile_pool(name="wB", bufs=1) as wb,
            tc.tile_pool(name="workB", bufs=1) as wk,
        ):
            w1 = wb.tile([128, KC * FFN], BF16, name="w_W1")
            nc.sync.dma_start(
                out=w1[:].rearrange("p (c n) -> p c n", n=FFN),
                in_=io["W1"].rearrange("(c p) n -> p c n", p=128))

            for blk in range(nblk):
                t0 = blk * TB
                xb = wk.tile([128, KC * TB], BF16, tag="xb", bufs=1, name="xb")
                nc.sync.dma_start(
                    out=xb[:].rearrange("p (c t) -> p c t", t=TB),
                    in_=io["xs"].rearrange("(c p) t -> p c t",
                                           p=128)[:, :, t0:t0 + TB])
                hb = wk.tile([128, MC1 * TB], BF16, tag="hb", bufs=1, name="hb")
                for mc in range(MC1):
                    ph = pp.tile([128, TB], F32, tag="acc", bufs=4, name="ph")
                    for kc in range(KC):
                        te.matmul(ph[:],
                                  w1[:, kc * FFN + mc * 128:
                                     kc * FFN + mc * 128 + 128],
                                  xb[:, kc * TB:(kc + 1) * TB],
                                  start=(kc == 0), stop=(kc == KC - 1))
                    s.activation(hb[:, mc * TB:(mc + 1) * TB], ph[:], AF.Relu,
                                 bias=C["b1"][:, mc:mc + 1])

                x2 = wk.tile([128, KC * TB], F32, tag="x2", bufs=1, name="x2")
                for mc in range(KC):
                    w2t = wk.tile([128, MC1 * 128], BF16, tag="w2t", bufs=2,
                                  name="w2t")
                    nc.sync.dma_start(
                        out=w2t[:].rearrange("p (c n) -> p c n", n=128),
                        in_=io["W2"].rearrange("(c p) n -> p c n",
                                               p=128)[:, :,
                                                      mc * 128:(mc + 1) * 128])
                    pf = pp.tile([128, TB], F32, tag="acc", bufs=4, name="pf")
                    for kc in range(MC1):
                        te.matmul(pf[:], w2t[:, kc * 128:(kc + 1) * 128],
                                  hb[:, kc * TB:(kc + 1) * TB],
                                  start=(kc == 0), stop=(kc == MC1 - 1))
                    tb2 = wk.tile([128, TB], F32, tag="tb", bufs=2, name="tb2")
                    s.activation(tb2[:], pf[:], AF.Identity,
                                 bias=C["b2"][:, mc:mc + 1])
                    v.tensor_add(x2[:, mc * TB:(mc + 1) * TB], tb2[:],
                                 xb[:, mc * TB:(mc + 1) * TB])

                yb = wk.tile([128, KC * TB], BF16, tag="yb", bufs=1, name="yb")
                _ln_apply(nc, pp, wk, x2, yb, C["g2"], C["be2"], C)

                # logits: one single-row matmul accumulation per class so
                # every scalar row lives at partition base 0.
                zc, ec = [], []
                for c in range(3):
                    pzc = pp.tile([1, TB], F32, tag="red", bufs=3,
                                  name=f"pz{c}")
                    for kc in range(KC):
                        te.matmul(pzc[:],
                                  C["Wwt"][:, kc * 3 + c:kc * 3 + c + 1],
                                  yb[:, kc * TB:(kc + 1) * TB],
                                  start=(kc == 0), stop=(kc == KC - 1))
                    zt = wk.tile([1, TB], F32, tag=f"z{c}", bufs=1,
                                 name=f"z{c}")
                    s.activation(zt[:], pzc[:], AF.Identity,
                                 bias=C["bwc"][:, c:c + 1])
                    zc.append(zt[:])
                mx3 = wk.tile([1, TB], F32, tag="mx3", bufs=1, name="mx3")[:]
                ss = wk.tile([1, TB], F32, tag="ss", bufs=1, name="ss")[:]
                rr = wk.tile([1, TB], F32, tag="rr", bufs=1, name="rr")[:]
                v.tensor_max(mx3, zc[0], zc[1])
                v.tensor_max(mx3, mx3, zc[2])
                for c in range(3):
                    et = wk.tile([1, TB], F32, tag=f"e{c}", bufs=1,
                                 name=f"e{c}")
                    v.tensor_sub(et[:], zc[c], mx3)
                    s.activation(et[:], et[:], AF.Exp)
                    ec.append(et[:])
                v.tensor_add(ss, ec[0], ec[1])
                v.tensor_add(ss, ss, ec[2])
                v.reciprocal(rr, ss)
                for c in range(3):
                    pt = wk.tile([1, TB], F32, tag=f"p{c}", bufs=1,
                                 name=f"p{c}")
                    v.tensor_mul(pt[:], ec[c], rr)
                    nc.sync.dma_start(
                        out=io["out"][t0:t0 + TB, c:c + 1].rearrange(
                            "t c -> c t"),
                        in_=pt[:])


def build_program(tpc=TPC):
    nc = bacc.Bacc("TRN2", target_bir_lowering=False, debug=False)
    io = {}

    def din(name, shape, dtype):
        io[name] = nc.dram_tensor(name, shape, dtype, kind="ExternalInput").ap()

    for name in ("m0", "m1", "m2", "dom"):
        din(name, [DIM, tpc], BF16)
    for name in ("Wg", "Wq", "Wk", "Wv"):
        din(name, [DIM, DIM], BF16)
    din("W1", [DIM, FFN], BF16)
    din("W2", [FFN, DIM], BF16)
    din("Ssel", [128, 128], BF16)
    din("Eexp", [16, 1024], BF16)
    din("onec", [128, 1], F32)
    din("onecb", [128, 1], BF16)
    din("oner", [1, 128], F32)
    for name, w in (("bg", KC), ("bq", KC), ("bk", KC), ("bv", KC),
                    ("b1", MC1), ("b2", KC), ("g1", KC), ("be1", KC),
                    ("g2", KC), ("be2", KC)):
        din(name, [128, w], F32)
    din("Wwt", [128, 3 * KC], BF16)
    din("bwc", [1, 3], F32)
    din("epsc", [1, 1], F32)
    io["xs"] = nc.dram_tensor("xs", [DIM, tpc], BF16).ap()
    io["out"] = nc.dram_tensor("out", [tpc, 3], F32,
                               kind="ExternalOutput").ap()

    with tile.TileContext(nc) as tc:
        _emit(nc, tc, io, tpc)
    nc.compile()
    return nc


def _chunk_cols(vec, width):
    """[width*128] host vector -> [128, width] chunk-column layout."""
    return np.ascontiguousarray(vec.reshape(width, 128).T).astype(np.float32)


def prep_host_inputs(inputs, tpc=TPC, ncores=NCORES):
    """Build per-core input maps (host-side shard + transpose + bf16 cast)."""
    bf = ml_dtypes.bfloat16
    f32 = np.float32

    def fm(x):  # [B, DIM] -> [DIM, B] bf16 feature-major
        return np.ascontiguousarray(np.asarray(x, f32).T.astype(bf))

    m0 = fm(inputs["m0"]); m1 = fm(inputs["m1"]); m2 = fm(inputs["m2"])
    dom = fm(inputs["domain_rep"])

    # head-selector S[p, c*16+h] and expander E[h, c*128+p]
    head_of = np.arange(DIM) // HD
    S = np.zeros((128, 128), f32)
    E = np.zeros((16, 1024), f32)
    for c in range(KC):
        for p in range(128):
            h = head_of[c * 128 + p]
            S[p, c * 16 + h] = 1.0
            E[h, c * 128 + p] = 1.0

    consts = {
        "Wg": (np.asarray(inputs["Wg"], f32) / 3.0).astype(bf),
        "Wq": (np.asarray(inputs["Wq"], f32) / np.sqrt(HD)).astype(bf),
        "Wk": np.asarray(inputs["Wk"], f32).astype(bf),
        "Wv": np.asarray(inputs["Wv"], f32).astype(bf),
        "W1": np.asarray(inputs["W1"], f32).astype(bf),
        "W2": np.asarray(inputs["W2"], f32).astype(bf),
        "Ssel": S.astype(bf),
        "Eexp": E.astype(bf),
        "onec": np.ones((128, 1), f32),
        "onecb": np.ones((128, 1), f32).astype(bf),
        "oner": np.ones((1, 128), f32),
        "bg": _chunk_cols(np.asarray(inputs["bg"], f32), KC),
        "bq": _chunk_cols(np.asarray(inputs["bq"], f32) / np.sqrt(HD), KC),
        "bk": _chunk_cols(np.asarray(inputs["bk"], f32), KC),
        "bv": _chunk_cols(np.asarray(inputs["bv"], f32), KC),
        "b1": _chunk_cols(np.asarray(inputs["b1"], f32), MC1),
        "b2": _chunk_cols(np.asarray(inputs["b2"], f32), KC),
        "g1": _chunk_cols(np.asarray(inputs["g1"], f32), KC),
        "be1": _chunk_cols(np.asarray(inputs["beta1"], f32), KC),
        "g2": _chunk_cols(np.asarray(inputs["g2"], f32), KC),
        "be2": _chunk_cols(np.asarray(inputs["beta2"], f32), KC),
        "Wwt": np.ascontiguousarray(
            np.asarray(inputs["Ww"], f32).reshape(KC, 128, 3)
            .transpose(1, 0, 2).reshape(128, 3 * KC)).astype(bf),
        "bwc": np.asarray(inputs["bw"], f32).reshape(1, 3),
        "epsc": np.full((1, 1), EPS, f32),
    }

    in_maps = []
    for c in range(ncores):
        sl = slice(c * tpc, (c + 1) * tpc)
        m = dict(consts)
        m["m0"] = np.ascontiguousarray(m0[:, sl])
        m["m1"] = np.ascontiguousarray(m1[:, sl])
        m["m2"] = np.ascontiguousarray(m2[:, sl])
        m["dom"] = np.ascontiguousarray(dom[:, sl])
        in_maps.append(m)
    return in_maps


def kernel(**inputs):
    from concourse.bass_utils import run_bass_kernel_spmd
    nc = build_program()
    in_maps = prep_host_inputs(inputs)
    res = run_bass_kernel_spmd(nc, in_maps, list(range(NCORES)))
    out = np.concatenate([res.results[c]["out"] for c in range(NCORES)],
                         axis=0)
    return np.ascontiguousarray(out.astype(np.float32))


# revision 20
# speedup vs baseline: 56.2218x; 56.2218x over previous
"""Trainium2 Bass kernel for the fused 3-modality attention + FFN + softmax model.

Layout strategy: pure data parallel over 8 NeuronCores (batch sharded), all
activations kept FEATURE-MAJOR on chip ([1024 feats = 8 chunks x 128
partitions, tokens in the free dim]) so no on-device transposes are needed.
All GEMMs run in bf16 (1 cycle/row on the PE) accumulating fp32 in PSUM.
LayerNorm reductions over features are ones-vector matmuls on the PE;
per-token scalars are broadcast back across partitions with K=1 expand
matmuls. Host-side prep: transpose activations to [DIM, B] bf16, pre-scale
Wg by 1/3 (modality mean) and Wq/bq by 1/sqrt(HD) (attention scale).
"""

import numpy as np
import ml_dtypes

import concourse.bacc as bacc
import concourse.bass as bass
import concourse.mybir as mybir
import concourse.tile as tile

B, DIM, H, FFN, HD = 16384, 1024, 16, 4096, 64
NCORES = 8
TPC = B // NCORES          # tokens per core
TB = 512                   # token block (matmul moving dim)
KC = DIM // 128            # 8 feature chunks
MC1 = FFN // 128           # 32 ffn chunks
EPS = 1e-5

BF16 = mybir.dt.bfloat16
F32 = mybir.dt.float32
F32R = mybir.dt.float32r
AF = mybir.ActivationFunctionType


def _ln_apply(nc, pp, wk, src_f32, out_bf, g, be, C, cbufs=2):
    """LayerNorm over features (partition x chunk axis); src modified in place.

    src_f32: [128, KC*TB] fp32 tile, out_bf: [128, KC*TB] bf16 tile.
    Per-token scalars live as rows of one packed [8, TB] fp32 tile.
    """
    v, s, te = nc.vector, nc.scalar, nc.tensor
    # bf16 copy of src for the (cheap, 1 cyc/row) column-sum matmuls
    xbc = wk.tile([128, KC * TB], BF16, tag="a4", bufs=cbufs, name="xbc")
    s.activation(xbc[:], src_f32[:], AF.Copy)
    pr1 = pp.tile([16, TB], F32, tag="red", bufs=3, name="pr1")
    for kc in range(KC):
        te.matmul(pr1[0:1, :], C["onecb"][:],
                  xbc[:, kc * TB:(kc + 1) * TB],
                  start=(kc == 0), stop=(kc == KC - 1))
    sq = wk.tile([128, KC * TB], BF16, tag="qb", bufs=1, name="sq")
    s.activation(sq[:], src_f32[:], AF.Square)
    pr2 = pp.tile([16, TB], F32, tag="red", bufs=3, name="pr2")
    for kc in range(KC):
        te.matmul(pr2[0:1, :], C["onecb"][:], sq[:, kc * TB:(kc + 1) * TB],
                  start=(kc == 0), stop=(kc == KC - 1))
    # per-token scalars: separate base-0 tiles (partition-alignment rules)
    mub = wk.tile([1, TB], BF16, tag="ln_mub", bufs=1, name="mub")[:]
    ex2 = wk.tile([1, TB], F32, tag="ln_ex2", bufs=1, name="ex2")[:]
    mu2 = wk.tile([1, TB], F32, tag="ln_mu2", bufs=1, name="mu2")[:]
    var = wk.tile([1, TB], F32, tag="ln_var", bufs=1, name="var")[:]
    sd = wk.tile([1, TB], F32, tag="ln_sd", bufs=1, name="sd")[:]
    rs = wk.tile([1, TB], F32, tag="ln_rs", bufs=1, name="rs")[:]
    rsb = wk.tile([1, TB], BF16, tag="ln_rsb", bufs=1, name="rsb")[:]
    s.activation(mub, pr1[0:1, :], AF.Copy, scale=1.0 / DIM)
    s.activation(ex2, pr2[0:1, :], AF.Copy, scale=1.0 / DIM)
    s.activation(mu2, mub, AF.Square)
    v.tensor_sub(var, ex2, mu2)
    s.activation(sd, var, AF.Sqrt, bias=C["epsc"][:])
    v.reciprocal(rs, sd)
    s.activation(rsb, rs, AF.Copy)
    pmu = pp.tile([128, TB], F32, tag="acc", bufs=5, name="pmu")
    te.matmul(pmu[:], C["onerb"][:], mub, start=True, stop=True)
    prs = pp.tile([128, TB], F32, tag="acc", bufs=5, name="prs")
    te.matmul(prs[:], C["onerb"][:], rsb, start=True, stop=True)
    mus = wk.tile([128, TB], F32, tag="mus", bufs=1, name="mus")
    v.tensor_copy(mus[:], pmu[:])
    rss = wk.tile([128, TB], F32, tag="rss", bufs=1, name="rss")
    v.tensor_copy(rss[:], prs[:])
    for kc in range(KC):
        sl = src_f32[:, kc * TB:(kc + 1) * TB]
        v.tensor_sub(sl, sl, mus[:])
        v.tensor_mul(sl, sl, rss[:])
        s.activation(out_bf[:, kc * TB:(kc + 1) * TB], sl, AF.Identity,
                     scale=g[:, kc:kc + 1], bias=be[:, kc:kc + 1])


def _emit(nc, tc, io, tpc):
    nblk = tpc // TB
    v, s, te = nc.vector, nc.scalar, nc.tensor

    with (
        tc.tile_pool(name="consts", bufs=1) as cp,
        tc.tile_pool(name="psum", bufs=1, space="PSUM") as pp,
    ):
        # ---- constants / small params ----
        C = {}
        for name, shape, dtype in (
            ("Ssel", [128, 128], BF16), ("Eexp", [16, 1024], BF16),
            ("onecb", [128, 1], BF16), ("onerb", [1, 128], BF16),
            ("bg", [128, KC], F32), ("bq", [128, KC], F32),
            ("bk", [128, KC], F32), ("bv", [128, KC], F32),
            ("b1", [128, MC1], F32), ("b2", [128, KC], F32),
            ("g1", [128, KC], F32), ("be1", [128, KC], F32),
            ("g2", [128, KC], F32), ("be2", [128, KC], F32),
            ("Wwt", [128, 3 * KC], BF16), ("bwc", [1, 3], F32),
            ("epsc", [1, 1], F32),
        ):
            t = cp.tile(shape, dtype, name=f"c_{name}")
            nc.sync.dma_start(out=t[:], in_=io[name])
            C[name] = t

        # ---------------- phase A: attention + LN1 ----------------
        with (
            tc.tile_pool(name="wA", bufs=1) as wa,
            tc.tile_pool(name="workA", bufs=1) as wk,
        ):
            wmap = {}
            for wn in ("Wg", "Wq", "Wk", "Wv"):
                wt = wa.tile([128, KC * DIM], BF16, name=f"w_{wn}")
                nc.sync.dma_start(
                    out=wt[:].rearrange("p (c n) -> p c n", n=DIM),
                    in_=io[wn].rearrange("(c p) n -> p c n", p=128))
                wmap[wn] = wt

            def wsl(wn, kc, mc):
                return wmap[wn][:, kc * DIM + mc * 128:kc * DIM + mc * 128 + 128]

            for blk in range(nblk):
                t0 = blk * TB
                ins = {}
                for name in ("m0", "m1", "m2", "dom"):
                    t = wk.tile([128, KC * TB], BF16, tag=f"in_{name}",
                                bufs=1, name=f"{name}_sb")
                    nc.sync.dma_start(
                        out=t[:].rearrange("p (c t) -> p c t", t=TB),
                        in_=io[name].rearrange("(c p) t -> p c t",
                                               p=128)[:, :, t0:t0 + TB])
                    ins[name] = t
                mj = [ins["m0"], ins["m1"], ins["m2"]]
                dom = ins["dom"]

                avg = wk.tile([128, KC * TB], BF16, tag="a4", bufs=2, name="avg")
                v.tensor_add(avg[:], mj[0][:], mj[1][:])
                v.tensor_add(avg[:], avg[:], mj[2][:])

                # global_rep -> qin: (psum + bg) + dom fused on the DVE
                qin = wk.tile([128, KC * TB], BF16, tag="a4", bufs=2, name="qin")
                for mc in range(KC):
                    pg = pp.tile([128, TB], F32, tag="acc", bufs=5, name="pg")
                    for kc in range(KC):
                        te.matmul(pg[:], wsl("Wg", kc, mc),
                                  avg[:, kc * TB:(kc + 1) * TB],
                                  start=(kc == 0), stop=(kc == KC - 1))
                    v.scalar_tensor_tensor(
                        qin[:, mc * TB:(mc + 1) * TB], pg[:],
                        C["bg"][:, mc:mc + 1], dom[:, mc * TB:(mc + 1) * TB],
                        mybir.AluOpType.add, mybir.AluOpType.add)

                qb = wk.tile([128, KC * TB], BF16, tag="qb", bufs=1, name="qb")
                for mc in range(KC):
                    pq = pp.tile([128, TB], F32, tag="acc", bufs=5, name="pq")
                    for kc in range(KC):
                        te.matmul(pq[:], wsl("Wq", kc, mc),
                                  qin[:, kc * TB:(kc + 1) * TB],
                                  start=(kc == 0), stop=(kc == KC - 1))
                    s.activation(qb[:, mc * TB:(mc + 1) * TB], pq[:],
                                 AF.Identity, bias=C["bq"][:, mc:mc + 1])

                # scores[h,t] per modality (Wq/bq pre-scaled by 1/sqrt(HD))
                sc = wk.tile([16, 3 * TB], F32, tag="sc", bufs=1,
                             name="sc")
                for j in range(3):
                    for mc in range(KC):
                        pk = pp.tile([128, TB], F32, tag="acc", bufs=5,
                                     name="pk")
                        for kc in range(KC):
                            te.matmul(pk[:], wsl("Wk", kc, mc),
                                      mj[j][:, kc * TB:(kc + 1) * TB],
                                      start=(kc == 0), stop=(kc == KC - 1))
                        tm = wk.tile([128, TB], BF16, tag="tm", bufs=2,
                                     name="tm")
                        v.scalar_tensor_tensor(
                            tm[:], pk[:], C["bk"][:, mc:mc + 1],
                            qb[:, mc * TB:(mc + 1) * TB],
                            mybir.AluOpType.add, mybir.AluOpType.mult)
                        ps = pp.tile([16, TB], F32, tag="red", bufs=3,
                                     name="ps")
                        te.matmul(ps[:], C["Ssel"][:, mc * 16:(mc + 1) * 16],
                                  tm[:], start=True, stop=True)
                        scj = sc[:, j * TB:(j + 1) * TB]
                        if mc == 0:
                            v.tensor_copy(scj, ps[:])
                        else:
                            v.tensor_add(scj, scj, ps[:])

                # softmax over the 3 modalities (all tiles at base 0)
                mx = wk.tile([16, TB], F32, tag="mx", bufs=1, name="mx")[:]
                sm = wk.tile([16, TB], F32, tag="sm", bufs=1, name="sm")[:]
                rc = wk.tile([16, TB], F32, tag="rc", bufs=1, name="rc")[:]
                v.tensor_max(mx, sc[:, 0:TB], sc[:, TB:2 * TB])
                v.tensor_max(mx, mx, sc[:, 2 * TB:3 * TB])
                for j in range(3):
                    scj = sc[:, j * TB:(j + 1) * TB]
                    v.tensor_sub(scj, scj, mx)
                ee = wk.tile([16, 3 * TB], F32, tag="ee", bufs=1, name="ee")
                s.activation(ee[:], sc[:], AF.Exp)
                v.tensor_add(sm, ee[:, 0:TB], ee[:, TB:2 * TB])
                v.tensor_add(sm, sm, ee[:, 2 * TB:3 * TB])
                v.reciprocal(rc, sm)
                ab = wk.tile([16, 3 * TB], BF16, tag="ab", bufs=1, name="ab")
                for j in range(3):
                    v.tensor_mul(ab[:, j * TB:(j + 1) * TB],
                                 ee[:, j * TB:(j + 1) * TB], rc)

                # attnout = sum_j bcast(attn_j) * (m_j @ Wv); bv folds to
                # +bv since sum_j attn_j = 1. All 24 v-matmuls for a chunk
                # are emitted before the attn-dependent expands so the PE
                # never stalls waiting for the softmax.
                xp = wk.tile([128, KC * TB], F32, tag="xp", bufs=1, name="xp")
                for mc in range(KC):
                    pvs = []
                    for j in range(3):
                        pv = pp.tile([128, TB], F32, tag="acc", bufs=5,
                                     name=f"pv{j}")
                        for kc in range(KC):
                            te.matmul(pv[:], wsl("Wv", kc, mc),
                                      mj[j][:, kc * TB:(kc + 1) * TB],
                                      start=(kc == 0), stop=(kc == KC - 1))
                        pvs.append(pv)
                    acc = wk.tile([128, TB], F32, tag="acc_s", bufs=2,
                                  name="acc")
                    for j in range(3):
                        vt = wk.tile([128, TB], BF16, tag="vt", bufs=2,
                                     name="vt")
                        s.activation(vt[:], pvs[j][:], AF.Copy)
                        pa = pp.tile([128, TB], F32, tag="red", bufs=3,
                                     name="pa")
                        te.matmul(pa[:], C["Eexp"][:, mc * 128:(mc + 1) * 128],
                                  ab[:, j * TB:(j + 1) * TB],
                                  start=True, stop=True)
                        if j == 0:
                            v.tensor_mul(acc[:], pa[:], vt[:])
                        else:
                            t2 = wk.tile([128, TB], F32, tag="t2", bufs=2,
                                         name="t2")
                            v.tensor_mul(t2[:], pa[:], vt[:])
                            v.tensor_add(acc[:], acc[:], t2[:])
                    v.scalar_tensor_tensor(
                        xp[:, mc * TB:(mc + 1) * TB], acc[:],
                        C["bv"][:, mc:mc + 1], dom[:, mc * TB:(mc + 1) * TB],
                        mybir.AluOpType.add, mybir.AluOpType.add)

                x_bf = wk.tile([128, KC * TB], BF16, tag="xbf", bufs=2,
                               name="x_bf")
                _ln_apply(nc, pp, wk, xp, x_bf, C["g1"], C["be1"], C)
                nc.sync.dma_start(
                    out=io["xs"].rearrange("(c p) t -> p c t",
                                           p=128)[:, :, t0:t0 + TB],
                    in_=x_bf[:].rearrange("p (c t) -> p c t", t=TB))

        # ---------------- phase B: FFN + LN2 + logits ----------------
        with (
            tc.tile_pool(name="wB", bufs=1) as wb,
            tc.tile_pool(name="workB", bufs=1) as wk,
        ):
            w1k = []
            for kc in range(KC):
                t = wb.tile([128, FFN], BF16, name=f"w_W1_{kc}")
                nc.sync.dma_start(
                    out=t[:],
                    in_=io["W1"].rearrange("(c p) n -> p c n",
                                           p=128)[:, kc, :])
                w1k.append(t)

            for blk in range(nblk):
                t0 = blk * TB
                xb = wk.tile([128, KC * TB], BF16, tag="xb", bufs=1, name="xb")
                nc.sync.dma_start(
                    out=xb[:].rearrange("p (c t) -> p c t", t=TB),
                    in_=io["xs"].rearrange("(c p) t -> p c t",
                                           p=128)[:, :, t0:t0 + TB])
                hb = wk.tile([128, MC1 * TB], BF16, tag="hb", bufs=1, name="hb")
                for mc in range(MC1):
                    ph = pp.tile([128, TB], F32, tag="acc", bufs=5, name="ph")
                    for kc in range(KC):
                        te.matmul(ph[:],
                                  w1k[kc][:, mc * 128:mc * 128 + 128],
                                  xb[:, kc * TB:(kc + 1) * TB],
                                  start=(kc == 0), stop=(kc == KC - 1))
                    s.activation(hb[:, mc * TB:(mc + 1) * TB], ph[:], AF.Relu,
                                 bias=C["b1"][:, mc:mc + 1])

                x2 = wk.tile([128, KC * TB], F32, tag="x2", bufs=1, name="x2")
                for mc in range(KC):
                    w2t = wk.tile([128, MC1 * 128], BF16, tag="w2t", bufs=2,
                                  name="w2t")
                    nc.sync.dma_start(
                        out=w2t[:].rearrange("p (c n) -> p c n", n=128),
                        in_=io["W2"].rearrange("(c p) n -> p c n",
                                               p=128)[:, :,
                                                      mc * 128:(mc + 1) * 128])
                    pf = pp.tile([128, TB], F32, tag="acc", bufs=5, name="pf")
                    for kc in range(MC1):
                        te.matmul(pf[:], w2t[:, kc * 128:(kc + 1) * 128],
                                  hb[:, kc * TB:(kc + 1) * TB],
                                  start=(kc == 0), stop=(kc == MC1 - 1))
                    v.scalar_tensor_tensor(
                        x2[:, mc * TB:(mc + 1) * TB], pf[:],
                        C["b2"][:, mc:mc + 1], xb[:, mc * TB:(mc + 1) * TB],
                        mybir.AluOpType.add, mybir.AluOpType.add)

                yb = wk.tile([128, KC * TB], BF16, tag="yb", bufs=1, name="yb")
                _ln_apply(nc, pp, wk, x2, yb, C["g2"], C["be2"], C, cbufs=1)

                # logits: one single-row matmul accumulation per class so
                # every scalar row lives at partition base 0.
                zc, ec = [], []
                for c in range(3):
                    pzc = pp.tile([1, TB], F32, tag="red", bufs=3,
                                  name=f"pz{c}")
                    for kc in range(KC):
                        te.matmul(pzc[:],
                                  C["Wwt"][:, kc * 3 + c:kc * 3 + c + 1],
                                  yb[:, kc * TB:(kc + 1) * TB],
                                  start=(kc == 0), stop=(kc == KC - 1))
                    zt = wk.tile([1, TB], F32, tag=f"z{c}", bufs=1,
                                 name=f"z{c}")
                    s.activation(zt[:], pzc[:], AF.Identity,
                                 bias=C["bwc"][:, c:c + 1])
                    zc.append(zt[:])
                mx3 = wk.tile([1, TB], F32, tag="mx3", bufs=1, name="mx3")[:]
                ss = wk.tile([1, TB], F32, tag="ss", bufs=1, name="ss")[:]
                rr = wk.tile([1, TB], F32, tag="rr", bufs=1, name="rr")[:]
                v.tensor_max(mx3, zc[0], zc[1])
                v.tensor_max(mx3, mx3, zc[2])
                for c in range(3):
                    et = wk.tile([1, TB], F32, tag=f"e{c}", bufs=1,
                                 name=f"e{c}")
                    v.tensor_sub(et[:], zc[c], mx3)
                    s.activation(et[:], et[:], AF.Exp)
                    ec.append(et[:])
                v.tensor_add(ss, ec[0], ec[1])
                v.tensor_add(ss, ss, ec[2])
                v.reciprocal(rr, ss)
                for c in range(3):
                    pt = wk.tile([1, TB], F32, tag=f"p{c}", bufs=1,
                                 name=f"p{c}")
                    v.tensor_mul(pt[:], ec[c], rr)
                    nc.sync.dma_start(
                        out=io["out"][t0:t0 + TB, c:c + 1].rearrange(
                            "t c -> c t"),
                        in_=pt[:])


def build_program(tpc=TPC):
    nc = bacc.Bacc("TRN2", target_bir_lowering=False, debug=False)
    io = {}

    def din(name, shape, dtype):
        io[name] = nc.dram_tensor(name, shape, dtype, kind="ExternalInput").ap()

    for name in ("m0", "m1", "m2", "dom"):
        din(name, [DIM, tpc], BF16)
    for name in ("Wg", "Wq", "Wk", "Wv"):
        din(name, [DIM, DIM], BF16)
    din("W1", [DIM, FFN], BF16)
    din("W2", [FFN, DIM], BF16)
    din("Ssel", [128, 128], BF16)
    din("Eexp", [16, 1024], BF16)
    din("onecb", [128, 1], BF16)
    din("onerb", [1, 128], BF16)
    for name, w in (("bg", KC), ("bq", KC), ("bk", KC), ("bv", KC),
                    ("b1", MC1), ("b2", KC), ("g1", KC), ("be1", KC),
                    ("g2", KC), ("be2", KC)):
        din(name, [128, w], F32)
    din("Wwt", [128, 3 * KC], BF16)
    din("bwc", [1, 3], F32)
    din("epsc", [1, 1], F32)
    io["xs"] = nc.dram_tensor("xs", [DIM, tpc], BF16).ap()
    io["out"] = nc.dram_tensor("out", [tpc, 3], F32,
                               kind="ExternalOutput").ap()

    with tile.TileContext(nc) as tc:
        _emit(nc, tc, io, tpc)
    nc.compile()
    return nc


def _chunk_cols(vec, width):
    """[width*128] host vector -> [128, width] chunk-column layout."""
    return np.ascontiguousarray(vec.reshape(width, 128).T).astype(np.float32)


def prep_host_inputs(inputs, tpc=TPC, ncores=NCORES):
    """Build per-core input maps (host-side shard + transpose + bf16 cast)."""
    bf = ml_dtypes.bfloat16
    f32 = np.float32

    def fm(x):  # [B, DIM] -> [DIM, B] bf16 feature-major
        return np.ascontiguousarray(np.asarray(x, f32).T.astype(bf))

    m0 = fm(inputs["m0"]); m1 = fm(inputs["m1"]); m2 = fm(inputs["m2"])
    dom = fm(inputs["domain_rep"])

    # head-selector S[p, c*16+h] and expander E[h, c*128+p]
    head_of = np.arange(DIM) // HD
    S = np.zeros((128, 128), f32)
    E = np.zeros((16, 1024), f32)
    for c in range(KC):
        for p in range(128):
            h = head_of[c * 128 + p]
            S[p, c * 16 + h] = 1.0
            E[h, c * 128 + p] = 1.0

    consts = {
        "Wg": (np.asarray(inputs["Wg"], f32) / 3.0).astype(bf),
        "Wq": (np.asarray(inputs["Wq"], f32) / np.sqrt(HD)).astype(bf),
        "Wk": np.asarray(inputs["Wk"], f32).astype(bf),
        "Wv": np.asarray(inputs["Wv"], f32).astype(bf),
        "W1": np.asarray(inputs["W1"], f32).astype(bf),
        "W2": np.asarray(inputs["W2"], f32).astype(bf),
        "Ssel": S.astype(bf),
        "Eexp": E.astype(bf),
        "onecb": np.ones((128, 1), f32).astype(bf),
        "onerb": np.ones((1, 128), f32).astype(bf),
        "bg": _chunk_cols(np.asarray(inputs["bg"], f32), KC),
        "bq": _chunk_cols(np.asarray(inputs["bq"], f32) / np.sqrt(HD), KC),
        "bk": _chunk_cols(np.asarray(inputs["bk"], f32), KC),
        "bv": _chunk_cols(np.asarray(inputs["bv"], f32), KC),
        "b1": _chunk_cols(np.asarray(inputs["b1"], f32), MC1),
        "b2": _chunk_cols(np.asarray(inputs["b2"], f32), KC),
        "g1": _chunk_cols(np.asarray(inputs["g1"], f32), KC),
        "be1": _chunk_cols(np.asarray(inputs["beta1"], f32), KC),
        "g2": _chunk_cols(np.asarray(inputs["g2"], f32), KC),
        "be2": _chunk_cols(np.asarray(inputs["beta2"], f32), KC),
        "Wwt": np.ascontiguousarray(
            np.asarray(inputs["Ww"], f32).reshape(KC, 128, 3)
            .transpose(1, 0, 2).reshape(128, 3 * KC)).astype(bf),
        "bwc": np.asarray(inputs["bw"], f32).reshape(1, 3),
        "epsc": np.full((1, 1), EPS, f32),
    }

    in_maps = []
    for c in range(ncores):
        sl = slice(c * tpc, (c + 1) * tpc)
        m = dict(consts)
        m["m0"] = np.ascontiguousarray(m0[:, sl])
        m["m1"] = np.ascontiguousarray(m1[:, sl])
        m["m2"] = np.ascontiguousarray(m2[:, sl])
        m["dom"] = np.ascontiguousarray(dom[:, sl])
        in_maps.append(m)
    return in_maps


def kernel(**inputs):
    from concourse.bass_utils import run_bass_kernel_spmd
    nc = build_program()
    in_maps = prep_host_inputs(inputs)
    res = run_bass_kernel_spmd(nc, in_maps, list(range(NCORES)))
    out = np.concatenate([res.results[c]["out"] for c in range(NCORES)],
                         axis=0)
    return np.ascontiguousarray(out.astype(np.float32))
